# revision 7
# baseline (speedup 1.0000x reference)
"""Trainium2 Bass kernel for nn_DetectionHead: three 1x1 convs + bias.

reference: out_i = einsum("bchw,oc->bohw", feat_i, w_i) + b_i   (i = 0,1,2)

Strategy (data-parallel over 8 NeuronCores):
  - Shard batch (16) -> 2 images per core; replicate the small weights.
  - Host-side: pre-transpose weights into k-tile-packed [128, 14*85]
    layout so the device only does fast contiguous DMAs.
  - Per core: for each image / feature map / column-piece, DMA the feature
    slab [128, KT, piece] into SBUF, run KT accumulating matmuls per
    400-column chunk, then ScalarE activation(Identity, bias) drains
    PSUM->SBUF adding the bias, and the result slab is DMA'd back to HBM.

MM_MODE:
  "f32"  - exact fp32 matmul (4 PE cycles/row)
  "f32r" - fp32-replicated matmul (1 cycle/row for N>=256, ~1e-4 l2 rel err)
"""

import os
import numpy as np
from contextlib import ExitStack

import concourse.bass as bass
import concourse.mybir as mybir
import concourse.tile as tile
from concourse import bacc
from concourse.bass import ds
from concourse.bass_utils import run_bass_kernel_spmd

N_CORES = 8
BATCH = 16
BP = BATCH // N_CORES  # images per core
OUT = 85
NCH = 400  # matmul moving-dim chunk (>=256 keeps float32r at 1 cycle/row)

MM_MODE = os.environ.get("BASS_MM_MODE", "f32")

# (C, HW, piece_cols, n_ktiles) ; piece_cols divides HW, NCH divides piece_cols
FEATS = [
    (256, 6400, 3200, 2),
    (512, 1600, 1600, 4),
    (1024, 400, 400, 8),
]
SPATIAL = [(80, 80), (40, 40), (20, 20)]
KT_TOTAL = sum(kt for _, _, _, kt in FEATS)  # 14
KT_OFS = [0, 2, 6]  # k-tile offset of each feat in the packed weight

_F32 = mybir.dt.float32
_F32R = mybir.dt.float32r


def _build_program(mm_mode=MM_MODE):
    mm_dt = _F32R if mm_mode == "f32r" else _F32
    nc = bacc.Bacc(
        "TRN2",
        target_bir_lowering=False,
        debug=False,
        enable_asserts=False,
        num_devices=N_CORES,
    )
    f_aps = [
        nc.dram_tensor(f"f{i}", (BP, C, HW), _F32, kind="ExternalInput").ap()
        for i, (C, HW, _, _) in enumerate(FEATS)
    ]
    wt_ap = nc.dram_tensor("wt", (128, KT_TOTAL * OUT), _F32, kind="ExternalInput").ap()
    bias_ap = nc.dram_tensor("bias", (OUT, 3), _F32, kind="ExternalInput").ap()
    o_aps = [
        nc.dram_tensor(f"o{i}", (BP, OUT, HW), _F32, kind="ExternalOutput").ap()
        for i, (C, HW, _, _) in enumerate(FEATS)
    ]

    with tile.TileContext(nc) as tc, ExitStack() as ctx:
        wpool = ctx.enter_context(tc.tile_pool(name="w", bufs=1))
        fin = ctx.enter_context(tc.tile_pool(name="fin", bufs=2))
        fout = ctx.enter_context(tc.tile_pool(name="fout", bufs=2))
        pp = ctx.enter_context(tc.tile_pool(name="pp", bufs=8, space="PSUM"))

        wt_t = wpool.tile([128, KT_TOTAL * OUT], mm_dt, tag="wt")
        nc.scalar.dma_start(wt_t[:], wt_ap[:].bitcast(mm_dt))
        b_t = wpool.tile([OUT, 3], _F32, tag="bias")
        nc.scalar.dma_start(b_t[:], bias_ap[:])

        for b in range(BP):
            for i, (C, HW, PIECE, KT) in enumerate(FEATS):
                KTH = KT // 2  # k-tiles per load half
                for h in range(HW // PIECE):
                    # Load each unit in two k-halves so the PE can start on the
                    # first half while the second is still in flight (bounds the
                    # post-last-load compute chain at the kernel tail).
                    fts = []
                    for kh in range(2):
                        ft = fin.tile([128, KTH, PIECE], mm_dt, tag=f"fin{i}k{kh}")
                        src = f_aps[i][
                            b,
                            kh * KTH * 128 : (kh + 1) * KTH * 128,
                            h * PIECE : (h + 1) * PIECE,
                        ].rearrange("(t p) n -> p t n", p=128)
                        nc.sync.dma_start(ft[:], src.bitcast(mm_dt))
                        fts.append(ft)

                    ot = fout.tile([OUT, PIECE], _F32, tag=f"fout{i}")
                    for j in range(PIECE // NCH):
                        ps = pp.tile([OUT, NCH], _F32, tag="ps")
                        for t in range(KT):
                            nc.tensor.matmul(
                                ps[:],
                                wt_t[:, ds((KT_OFS[i] + t) * OUT, OUT)],
                                fts[t // KTH][:, t % KTH, ds(j * NCH, NCH)],
                                start=(t == 0),
                                stop=(t == KT - 1),
                            )
                        # Alternate the PSUM drain between ScalarE and VectorE
                        # so consecutive chunks drain in parallel instead of
                        # serializing on one engine at the kernel tail.
                        if j % 2 == 0:
                            nc.scalar.activation(
                                ot[:, ds(j * NCH, NCH)],
                                ps[:],
                                mybir.ActivationFunctionType.Identity,
                                bias=b_t[:, i : i + 1],
                            )
                        else:
                            nc.vector.tensor_scalar_add(
                                ot[:, ds(j * NCH, NCH)], ps[:], b_t[:, i : i + 1]
                            )
                    # Store in column sub-pieces so writeback starts while later
                    # chunks still compute. SWDGE (gpsimd) stores fan
                    # descriptors across the 16 SDMA engines; an HWDGE store of
                    # [85, N] emits only 85 row descriptors, which land on just
                    # 5 engines (17-descriptor packets) and serialize writeback.
                    # The tiny final o2 stores go on the HWDGE scalar ring
                    # instead: ~0.6us first-byte latency vs ~2us for SWDGE.
                    n_sub = 4 if i == 0 else 2
                    SP = PIECE // n_sub
                    for sh in range(n_sub):
                        dst = o_aps[i][
                            b, :, h * PIECE + sh * SP : h * PIECE + (sh + 1) * SP
                        ]
                        src_t = ot[:, ds(sh * SP, SP)]
                        if i == 2:
                            nc.scalar.dma_start(dst, src_t)
                        else:
                            nc.gpsimd.dma_start(dst, src_t)

    nc.compile()
    return nc


_CACHE: dict = {}


def _get_nc():
    if "nc" not in _CACHE:
        _CACHE["nc"] = _build_program()
    return _CACHE["nc"]


def _pack_weights(w0, w1, w2):
    blocks = []
    for w in (w0, w1, w2):
        c = w.shape[1]
        # wt_packed[p, t*85+o] = w[o, t*128+p]
        blocks.append(
            np.ascontiguousarray(
                w.T.reshape(c // 128, 128, OUT).transpose(1, 0, 2).reshape(128, -1)
            )
        )
    return np.concatenate(blocks, axis=1).astype(np.float32)


def kernel(feat0, feat1, feat2, w0, b0, w1, b1, w2, b2):
    feats = [np.asarray(f, dtype=np.float32) for f in (feat0, feat1, feat2)]
    ws = [np.asarray(w, dtype=np.float32) for w in (w0, w1, w2)]
    bs = [np.asarray(b, dtype=np.float32) for b in (b0, b1, b2)]

    nc = _get_nc()
    wt = _pack_weights(*ws)
    bias = np.stack(bs, axis=1).astype(np.float32)  # [85, 3]

    in_maps = []
    for c in range(N_CORES):
        m = {"wt": wt, "bias": bias}
        for i, (C, HW, _, _) in enumerate(FEATS):
            m[f"f{i}"] = np.ascontiguousarray(
                feats[i][c * BP : (c + 1) * BP].reshape(BP, C, HW)
            )
        in_maps.append(m)

    res = run_bass_kernel_spmd(nc, in_maps, core_ids=list(range(N_CORES)))
    _CACHE["last_results"] = res

    outs = []
    for i, (C, HW, _, _) in enumerate(FEATS):
        h, w = SPATIAL[i]
        full = np.concatenate([res.results[c][f"o{i}"] for c in range(N_CORES)], axis=0)
        outs.append(full.reshape(BATCH, OUT, h, w).astype(np.float32))
    return tuple(outs)


# revision 8
# speedup vs baseline: 1.0544x; 1.0544x over previous
"""Trainium2 Bass kernel for nn_DetectionHead: three 1x1 convs + bias.

reference: out_i = einsum("bchw,oc->bohw", feat_i, w_i) + b_i   (i = 0,1,2)

Strategy (data-parallel over 8 NeuronCores):
  - Shard batch (16) -> 2 images per core; replicate the small weights.
  - Host-side: pre-transpose weights into k-tile-packed [128, 14*85]
    layout so the device only does fast contiguous DMAs.
  - Per core: for each image / feature map / column-piece, DMA the feature
    slab [128, KT, piece] into SBUF, run KT accumulating matmuls per
    400-column chunk, then ScalarE activation(Identity, bias) drains
    PSUM->SBUF adding the bias, and the result slab is DMA'd back to HBM.

MM_MODE:
  "f32"  - exact fp32 matmul (4 PE cycles/row)
  "f32r" - fp32-replicated matmul (1 cycle/row for N>=256, ~1e-4 l2 rel err)
"""

import os
import numpy as np
from contextlib import ExitStack

import concourse.bass as bass
import concourse.mybir as mybir
import concourse.tile as tile
from concourse import bacc
from concourse.bass import ds
from concourse.bass_utils import run_bass_kernel_spmd

N_CORES = 8
BATCH = 16
BP = BATCH // N_CORES  # images per core
OUT = 85
NCH = 400  # matmul moving-dim chunk (>=256 keeps float32r at 1 cycle/row)

MM_MODE = os.environ.get("BASS_MM_MODE", "f32")

# (C, HW, piece_cols, n_ktiles) ; piece_cols divides HW, NCH divides piece_cols
FEATS = [
    (256, 6400, 3200, 2),
    (512, 1600, 1600, 4),
    (1024, 400, 400, 8),
]
SPATIAL = [(80, 80), (40, 40), (20, 20)]
KT_TOTAL = sum(kt for _, _, _, kt in FEATS)  # 14
KT_OFS = [0, 2, 6]  # k-tile offset of each feat in the packed weight

_F32 = mybir.dt.float32
_F32R = mybir.dt.float32r


def _build_program(mm_mode=MM_MODE):
    mm_dt = _F32R if mm_mode == "f32r" else _F32
    nc = bacc.Bacc(
        "TRN2",
        target_bir_lowering=False,
        debug=False,
        enable_asserts=False,
        num_devices=N_CORES,
    )
    f_aps = [
        nc.dram_tensor(f"f{i}", (BP, C, HW), _F32, kind="ExternalInput").ap()
        for i, (C, HW, _, _) in enumerate(FEATS)
    ]
    wt_ap = nc.dram_tensor("wt", (128, KT_TOTAL * OUT), _F32, kind="ExternalInput").ap()
    bias_ap = nc.dram_tensor("bias", (OUT, 3), _F32, kind="ExternalInput").ap()
    o_aps = [
        nc.dram_tensor(f"o{i}", (BP, OUT, HW), _F32, kind="ExternalOutput").ap()
        for i, (C, HW, _, _) in enumerate(FEATS)
    ]

    with tile.TileContext(nc) as tc, ExitStack() as ctx:
        wpool = ctx.enter_context(tc.tile_pool(name="w", bufs=1))
        fin = ctx.enter_context(tc.tile_pool(name="fin", bufs=2))
        fout = ctx.enter_context(tc.tile_pool(name="fout", bufs=2))
        pp = ctx.enter_context(tc.tile_pool(name="pp", bufs=8, space="PSUM"))

        wt_t = wpool.tile([128, KT_TOTAL * OUT], mm_dt, tag="wt")
        nc.scalar.dma_start(wt_t[:], wt_ap[:].bitcast(mm_dt))
        b_t = wpool.tile([OUT, 3], _F32, tag="bias")
        nc.scalar.dma_start(b_t[:], bias_ap[:])

        for b in range(BP):
            for i, (C, HW, PIECE, KT) in enumerate(FEATS):
                KTH = KT // 2  # k-tiles per load half
                for h in range(HW // PIECE):
                    # Load each unit in two k-halves so the PE can start on the
                    # first half while the second is still in flight (bounds the
                    # post-last-load compute chain at the kernel tail).
                    fts = []
                    for kh in range(2):
                        ft = fin.tile([128, KTH, PIECE], mm_dt, tag=f"fin{i}k{kh}")
                        src = f_aps[i][
                            b,
                            kh * KTH * 128 : (kh + 1) * KTH * 128,
                            h * PIECE : (h + 1) * PIECE,
                        ].rearrange("(t p) n -> p t n", p=128)
                        nc.sync.dma_start(ft[:], src.bitcast(mm_dt))
                        fts.append(ft)

                    ot = fout.tile([OUT, PIECE], _F32, tag=f"fout{i}")
                    for j in range(PIECE // NCH):
                        ps = pp.tile([OUT, NCH], _F32, tag="ps")
                        for t in range(KT):
                            nc.tensor.matmul(
                                ps[:],
                                wt_t[:, ds((KT_OFS[i] + t) * OUT, OUT)],
                                fts[t // KTH][:, t % KTH, ds(j * NCH, NCH)],
                                start=(t == 0),
                                stop=(t == KT - 1),
                            )
                        nc.scalar.activation(
                            ot[:, ds(j * NCH, NCH)],
                            ps[:],
                            mybir.ActivationFunctionType.Identity,
                            bias=b_t[:, i : i + 1],
                        )
                    # Store in column sub-pieces so writeback starts while later
                    # chunks still compute. SWDGE (gpsimd) stores fan
                    # descriptors across the 16 SDMA engines; an HWDGE store of
                    # [85, N] emits only 85 row descriptors, which land on just
                    # 5 engines (17-descriptor packets) and serialize writeback.
                    # The tiny final o2 stores go on the HWDGE scalar ring
                    # instead: ~0.6us first-byte latency vs ~2us for SWDGE.
                    n_sub = 4 if i == 0 else 2
                    SP = PIECE // n_sub
                    for sh in range(n_sub):
                        dst = o_aps[i][
                            b, :, h * PIECE + sh * SP : h * PIECE + (sh + 1) * SP
                        ]
                        src_t = ot[:, ds(sh * SP, SP)]
                        if i == 2:
                            nc.scalar.dma_start(dst, src_t)
                        else:
                            nc.gpsimd.dma_start(dst, src_t)

    nc.compile()
    return nc


_CACHE: dict = {}


def _get_nc():
    if "nc" not in _CACHE:
        _CACHE["nc"] = _build_program()
    return _CACHE["nc"]


def _pack_weights(w0, w1, w2):
    blocks = []
    for w in (w0, w1, w2):
        c = w.shape[1]
        # wt_packed[p, t*85+o] = w[o, t*128+p]
        blocks.append(
            np.ascontiguousarray(
                w.T.reshape(c // 128, 128, OUT).transpose(1, 0, 2).reshape(128, -1)
            )
        )
    return np.concatenate(blocks, axis=1).astype(np.float32)


def kernel(feat0, feat1, feat2, w0, b0, w1, b1, w2, b2):
    feats = [np.asarray(f, dtype=np.float32) for f in (feat0, feat1, feat2)]
    ws = [np.asarray(w, dtype=np.float32) for w in (w0, w1, w2)]
    bs = [np.asarray(b, dtype=np.float32) for b in (b0, b1, b2)]

    nc = _get_nc()
    wt = _pack_weights(*ws)
    bias = np.stack(bs, axis=1).astype(np.float32)  # [85, 3]

    in_maps = []
    for c in range(N_CORES):
        m = {"wt": wt, "bias": bias}
        for i, (C, HW, _, _) in enumerate(FEATS):
            m[f"f{i}"] = np.ascontiguousarray(
                feats[i][c * BP : (c + 1) * BP].reshape(BP, C, HW)
            )
        in_maps.append(m)

    res = run_bass_kernel_spmd(nc, in_maps, core_ids=list(range(N_CORES)))
    _CACHE["last_results"] = res

    outs = []
    for i, (C, HW, _, _) in enumerate(FEATS):
        h, w = SPATIAL[i]
        full = np.concatenate([res.results[c][f"o{i}"] for c in range(N_CORES)], axis=0)
        outs.append(full.reshape(BATCH, OUT, h, w).astype(np.float32))
    return tuple(outs)


# revision 9
# speedup vs baseline: 1.0965x; 1.0400x over previous
"""Trainium2 Bass kernel for nn_DetectionHead: three 1x1 convs + bias.

reference: out_i = einsum("bchw,oc->bohw", feat_i, w_i) + b_i   (i = 0,1,2)

Strategy (data-parallel over 8 NeuronCores):
  - Shard batch (16) -> 2 images per core; replicate the small weights.
  - Host-side: pre-transpose weights into k-tile-packed [128, 14*85]
    layout so the device only does fast contiguous DMAs.
  - Per core: for each image / feature map / column-piece, DMA the feature
    slab [128, KT, piece] into SBUF, run KT accumulating matmuls per
    400-column chunk, then ScalarE activation(Identity, bias) drains
    PSUM->SBUF adding the bias, and the result slab is DMA'd back to HBM.

MM_MODE:
  "f32"  - exact fp32 matmul (4 PE cycles/row)
  "f32r" - fp32-replicated matmul (1 cycle/row for N>=256, ~1e-4 l2 rel err)
"""

import os
import numpy as np
from contextlib import ExitStack

import concourse.bass as bass
import concourse.mybir as mybir
import concourse.tile as tile
from concourse import bacc
from concourse.bass import ds
from concourse.bass_utils import run_bass_kernel_spmd

N_CORES = 8
BATCH = 16
BP = BATCH // N_CORES  # images per core
OUT = 85
NCH = 400  # matmul moving-dim chunk (>=256 keeps float32r at 1 cycle/row)

MM_MODE = os.environ.get("BASS_MM_MODE", "f32")

# (C, HW, piece_cols, n_ktiles) ; piece_cols divides HW, NCH divides piece_cols
FEATS = [
    (256, 6400, 3200, 2),
    (512, 1600, 1600, 4),
    (1024, 400, 400, 8),
]
SPATIAL = [(80, 80), (40, 40), (20, 20)]
KT_TOTAL = sum(kt for _, _, _, kt in FEATS)  # 14
KT_OFS = [0, 2, 6]  # k-tile offset of each feat in the packed weight

_F32 = mybir.dt.float32
_F32R = mybir.dt.float32r


def _build_program(mm_mode=MM_MODE):
    mm_dt = _F32R if mm_mode == "f32r" else _F32
    nc = bacc.Bacc(
        "TRN2",
        target_bir_lowering=False,
        debug=False,
        enable_asserts=False,
        num_devices=N_CORES,
    )
    f_aps = [
        nc.dram_tensor(f"f{i}", (BP, C, HW), _F32, kind="ExternalInput").ap()
        for i, (C, HW, _, _) in enumerate(FEATS)
    ]
    wt_ap = nc.dram_tensor("wt", (128, KT_TOTAL * OUT), _F32, kind="ExternalInput").ap()
    bias_ap = nc.dram_tensor("bias", (OUT, 3), _F32, kind="ExternalInput").ap()
    o_aps = [
        nc.dram_tensor(f"o{i}", (BP, OUT, HW), _F32, kind="ExternalOutput").ap()
        for i, (C, HW, _, _) in enumerate(FEATS)
    ]

    with tile.TileContext(nc) as tc, ExitStack() as ctx:
        wpool = ctx.enter_context(tc.tile_pool(name="w", bufs=1))
        fin = ctx.enter_context(tc.tile_pool(name="fin", bufs=2))
        fout = ctx.enter_context(tc.tile_pool(name="fout", bufs=2))
        pp = ctx.enter_context(tc.tile_pool(name="pp", bufs=8, space="PSUM"))

        wt_t = wpool.tile([128, KT_TOTAL * OUT], mm_dt, tag="wt")
        nc.scalar.dma_start(wt_t[:], wt_ap[:].bitcast(mm_dt))
        b_t = wpool.tile([OUT, 3], _F32, tag="bias")
        nc.scalar.dma_start(b_t[:], bias_ap[:])

        for b in range(BP):
            for i, (C, HW, PIECE, KT) in enumerate(FEATS):
                KTH = KT // 2  # k-tiles per load half
                for h in range(HW // PIECE):
                    # Load each unit in two k-halves so the PE can start on the
                    # first half while the second is still in flight (bounds the
                    # post-last-load compute chain at the kernel tail).
                    fts = []
                    for kh in range(2):
                        ft = fin.tile([128, KTH, PIECE], mm_dt, tag=f"fin{i}k{kh}")
                        src = f_aps[i][
                            b,
                            kh * KTH * 128 : (kh + 1) * KTH * 128,
                            h * PIECE : (h + 1) * PIECE,
                        ].rearrange("(t p) n -> p t n", p=128)
                        nc.sync.dma_start(ft[:], src.bitcast(mm_dt))
                        fts.append(ft)

                    ot = fout.tile([OUT, PIECE], _F32, tag=f"fout{i}")
                    for j in range(PIECE // NCH):
                        ps = pp.tile([OUT, NCH], _F32, tag="ps")
                        for t in range(KT):
                            nc.tensor.matmul(
                                ps[:],
                                wt_t[:, ds((KT_OFS[i] + t) * OUT, OUT)],
                                fts[t // KTH][:, t % KTH, ds(j * NCH, NCH)],
                                start=(t == 0),
                                stop=(t == KT - 1),
                            )
                        nc.scalar.activation(
                            ot[:, ds(j * NCH, NCH)],
                            ps[:],
                            mybir.ActivationFunctionType.Identity,
                            bias=b_t[:, i : i + 1],
                        )
                    # Store in column sub-pieces so writeback starts while later
                    # chunks still compute. SWDGE (gpsimd) stores fan
                    # descriptors across the 16 SDMA engines; an HWDGE store of
                    # [85, N] emits only 85 row descriptors, which land on just
                    # 5 engines (17-descriptor packets) and serialize writeback.
                    # The tiny final o2 stores go on the HWDGE scalar ring
                    # instead: ~0.6us first-byte latency vs ~2us for SWDGE.
                    n_sub = 2
                    SP = PIECE // n_sub
                    for sh in range(n_sub):
                        dst = o_aps[i][
                            b, :, h * PIECE + sh * SP : h * PIECE + (sh + 1) * SP
                        ]
                        src_t = ot[:, ds(sh * SP, SP)]
                        if i == 2:
                            nc.scalar.dma_start(dst, src_t)
                        else:
                            nc.gpsimd.dma_start(dst, src_t)

    nc.compile()
    return nc


_CACHE: dict = {}


def _get_nc():
    if "nc" not in _CACHE:
        _CACHE["nc"] = _build_program()
    return _CACHE["nc"]


def _pack_weights(w0, w1, w2):
    blocks = []
    for w in (w0, w1, w2):
        c = w.shape[1]
        # wt_packed[p, t*85+o] = w[o, t*128+p]
        blocks.append(
            np.ascontiguousarray(
                w.T.reshape(c // 128, 128, OUT).transpose(1, 0, 2).reshape(128, -1)
            )
        )
    return np.concatenate(blocks, axis=1).astype(np.float32)


def kernel(feat0, feat1, feat2, w0, b0, w1, b1, w2, b2):
    feats = [np.asarray(f, dtype=np.float32) for f in (feat0, feat1, feat2)]
    ws = [np.asarray(w, dtype=np.float32) for w in (w0, w1, w2)]
    bs = [np.asarray(b, dtype=np.float32) for b in (b0, b1, b2)]

    nc = _get_nc()
    wt = _pack_weights(*ws)
    bias = np.stack(bs, axis=1).astype(np.float32)  # [85, 3]

    in_maps = []
    for c in range(N_CORES):
        m = {"wt": wt, "bias": bias}
        for i, (C, HW, _, _) in enumerate(FEATS):
            m[f"f{i}"] = np.ascontiguousarray(
                feats[i][c * BP : (c + 1) * BP].reshape(BP, C, HW)
            )
        in_maps.append(m)

    res = run_bass_kernel_spmd(nc, in_maps, core_ids=list(range(N_CORES)))
    _CACHE["last_results"] = res

    outs = []
    for i, (C, HW, _, _) in enumerate(FEATS):
        h, w = SPATIAL[i]
        full = np.concatenate([res.results[c][f"o{i}"] for c in range(N_CORES)], axis=0)
        outs.append(full.reshape(BATCH, OUT, h, w).astype(np.float32))
    return tuple(outs)
